# revision 47
# baseline (speedup 1.0000x reference)
"""Deformable Conv2D Trainium2 kernel.

Strategy (data-parallel over batch, 2 samples per core on 8 cores):
  - offset conv (3x3, C=64 -> 18) in split precision: x = xh + xl (bf16
    halves stacked on 128 partitions), w = wh + wl; psum accumulates
    xh@wh + xl@wh (K=128) per tap, plus xh@wl corrections with the
    (t, t+3) tap pairs K-stacked via a row-shifted copy of xh
    (needed: the reference's bilinear weights are discontinuous at the
    right/bottom clip edge, and the true offsets pass within 4e-5 of it).
  - the whole prep pipeline (offset conv, PE transpose to pixel-major,
    DVE index/weight prep, selector-matmul index wrap) runs at
    quarter-sample granularity (8 pixel blocks) so the first gather
    fires ~25us in and sample 1's prep hides under sample 0's stream.
  - bilinear gather: host-staged token table with one 512B token per
    (y0,x0) = the 2x2 corner patch [64 ch, 4 corners] in bf16
    (corner-minor so the combine mult runs in a packed DVE mode); a
    single SWDGE dma_gather per 256-pixel chunk fetches all 9 taps.
  - combine: one DVE mult (corner weights broadcast along channels) +
    one tensor_reduce over the 4 corners, bf16.
  - sampled taps are PE-transposed back to channel-major (tap pairs as
    single [128,128] transposes) and the 1x1 conv runs as 5 accumulated
    K=128 bf16 matmuls per 128-pixel block.
"""

import sys
import numpy as np

sys.path.insert(0, "/opt/trn_rl_repo")

B, H, W, C = 16, 64, 64, 64
K2, OFF_CH, F = 9, 18, 128
NPIX = H * W            # 4096
NBLK = NPIX // 128      # 32 pixel blocks per sample
SPB = 2                 # samples per core
NCORES = 8


def build_nc(reps=1, variant="full"):
    import concourse.tile as tile
    import concourse.mybir as mybir
    from concourse import bacc
    from concourse.masks import make_identity

    f32 = mybir.dt.float32
    bf = mybir.dt.bfloat16
    i16 = mybir.dt.int16
    i32 = mybir.dt.int32
    Alu = mybir.AluOpType
    Act = mybir.ActivationFunctionType
    Ax = mybir.AxisListType

    nc = bacc.Bacc(None, target_bir_lowering=False, num_swdge_queues=4)
    with tile.TileContext(nc) as tc:
        with tc.tile_pool(name="dram", bufs=1, space="DRAM") as dram:
            xcm_d = dram.tile([SPB, 128, 66 * 66], bf, kind="ExternalInput", name="xcm", uniquify=False)
            xcm2_d = dram.tile([SPB, 128, 66 * 66], bf, kind="ExternalInput", name="xcm2", uniquify=False)
            tok_d = dram.tile([SPB * NPIX, 256], bf, kind="ExternalInput", name="tok", uniquify=False)
            woff_d = dram.tile([128, K2, OFF_CH], bf, kind="ExternalInput", name="woff", uniquify=False)
            wloff_d = dram.tile([64, 3, OFF_CH], bf, kind="ExternalInput", name="wloff", uniquify=False)
            wlp_d = dram.tile([128, 3, OFF_CH], bf, kind="ExternalInput", name="wlp", uniquify=False)
            wconv_d = dram.tile([128, 5, F], bf, kind="ExternalInput", name="wconv5", uniquify=False)
            boff_d = dram.tile([OFF_CH, 1], f32, kind="ExternalInput", name="boff", uniquify=False)
            bconv_d = dram.tile([F, 1], f32, kind="ExternalInput", name="bconv", uniquify=False)
            ramp_d = dram.tile([128, NBLK, 2], f32, kind="ExternalInput", name="ramp", uniquify=False)
            sel_d = dram.tile([128, 8, 16], f32, kind="ExternalInput", name="sel", uniquify=False)
            out_d = dram.tile([SPB, F, NPIX], f32, kind="ExternalOutput", name="out", uniquify=False)

            with tc.tile_pool(name="const", bufs=1) as cpool, \
                 tc.tile_pool(name="sbuf", bufs=1) as pool, \
                 tc.tile_pool(name="psum", bufs=1, space="PSUM") as psum:
                woff = cpool.tile([128, K2, OFF_CH], bf)
                wloff = cpool.tile([64, 3, OFF_CH], bf)
                wlp = cpool.tile([128, 3, OFF_CH], bf)
                wconv = cpool.tile([128, 5, F], bf)
                boff = cpool.tile([OFF_CH, 1], f32)
                bconv = cpool.tile([F, 1], f32)
                ramp = cpool.tile([128, NBLK, 2], f32)
                sel = cpool.tile([128, 8, 16], f32)
                ident = cpool.tile([128, 128], f32)
                identb = cpool.tile([128, 128], bf)
                nc.sync.dma_start(woff[:], woff_d[:])
                nc.sync.dma_start(wloff[:], wloff_d[:])
                nc.sync.dma_start(wlp[:], wlp_d[:])
                nc.sync.dma_start(wconv[:], wconv_d[:])
                nc.sync.dma_start(boff[:], boff_d[:])
                nc.sync.dma_start(bconv[:], bconv_d[:])
                nc.sync.dma_start(ramp[:], ramp_d[:])
                nc.sync.dma_start(sel[:], sel_d[:])
                make_identity(nc, ident[:])
                make_identity(nc, identb[:])
                if variant != "full":
                    t2full = cpool.tile([128, 18 * 256], bf)
                    nc.gpsimd.memset(t2full[:], 0.5)
                c63 = cpool.tile([128, 1], f32)
                nc.gpsimd.memset(c63[:], 63.0)

                warm = psum.tile([128, 5, 128], bf, tag="pt5", bufs=2, name="warm")
                for i in range(96):
                    nc.tensor.transpose(warm[:, 0, :], identb[:], identb[:])

                for rep in range(reps):
                  T = {}
                  for s in range(SPB):
                      T[s] = dict(
                          xcm=pool.tile([128, 66 * 66], bf, tag="xcm", bufs=2, name=f"xcm{rep}_{s}"),
                          xcm2=pool.tile([128, 66 * 66], bf, tag="xcm2", bufs=2, name=f"xcm2_{rep}_{s}"),
                          off_cm=pool.tile([OFF_CH, NPIX], f32, tag="offcm", bufs=2, name=f"offcm{rep}_{s}"),
                          off_pm=pool.tile([128, NBLK, OFF_CH], f32, tag="offpm", bufs=2, name=f"offpm{rep}_{s}"),
                          nxy=pool.tile([128, NBLK, OFF_CH], f32, tag="nxy", bufs=2, name=f"nxy{rep}_{s}"),
                          frac=pool.tile([128, NBLK, OFF_CH], f32, tag="frac", bufs=2, name=f"frac{rep}_{s}"),
                          f0=pool.tile([128, NBLK, OFF_CH], f32, tag="f0", bufs=2, name=f"f0{rep}_{s}"),
                          w0=pool.tile([128, NBLK, OFF_CH], f32, tag="w0", bufs=2, name=f"w0{rep}_{s}"),
                          nxyi=pool.tile([128, NBLK, OFF_CH], i32, tag="nxyi", bufs=2, name=f"nxyi{rep}_{s}"),
                          w4=pool.tile([128, NBLK, K2, 4], bf, tag="w4", bufs=2, name=f"w4{rep}_{s}"),
                          tokf=pool.tile([128, NBLK, K2], f32, tag="tokf", bufs=2, name=f"tokf{rep}_{s}"),
                          idxw=pool.tile([128, NBLK, K2, 8], i16, tag="idxw", bufs=2, name=f"idxw{rep}_{s}"),
                      )

                  def load_x(s):
                      nc.sync.dma_start(T[s]["xcm"][:], xcm_d[s])
                      nc.sync.dma_start(T[s]["xcm2"][:], xcm2_d[s])

                  def emit_ph1_chunk(s, ch):
                      # -------- phase 1: offset conv, one 512-px chunk ------
                      xcm3 = T[s]["xcm"].rearrange("c (a b) -> c a b", b=66)
                      xcm23 = T[s]["xcm2"].rearrange("c (a b) -> c a b", b=66)
                      off_cm = T[s]["off_cm"]
                      poff = psum.tile([OFF_CH, 512], f32, tag="poff", bufs=2, name=f"poff{rep}_{s}_{ch}")
                      for tap in range(K2):
                          dy, dx = tap // 3, tap % 3
                          rhs = xcm3[:, ch * 8 + dy: ch * 8 + dy + 8, dx: dx + 64]
                          nc.tensor.matmul(poff[:], woff[:, tap, :], rhs,
                                           start=(tap == 0), stop=False)
                      # low-order corrections: taps (t, t+3) K-stacked via
                      # the one-row-shifted xh copy, taps 6..8 single.
                      for t in range(3):
                          rhs = xcm23[:, ch * 8: ch * 8 + 8, t: t + 64]
                          nc.tensor.matmul(poff[:], wlp[:, t, :], rhs,
                                           start=False, stop=False)
                      for t in range(3):
                          rhs = xcm3[0:64, ch * 8 + 2: ch * 8 + 2 + 8, t: t + 64]
                          nc.tensor.matmul(poff[:], wloff[:, t, :], rhs,
                                           start=False, stop=(t == 2))
                      nc.scalar.activation(off_cm[:, ch * 512:(ch + 1) * 512], poff[:],
                                           Act.Identity, bias=boff[:], scale=1.0)

                  def emit_ph2_group(s, b0, gsz):
                      # -------- phase 2: transpose gsz blocks at b0 ---------
                      off_cm = T[s]["off_cm"]
                      pofft = psum.tile([128, 8 * OFF_CH], f32, tag="pofft", bufs=1, name=f"pofft{rep}_{s}_{b0}")
                      for bi in range(gsz):
                          b = b0 + bi
                          nc.tensor.transpose(
                              pofft[:, bi * OFF_CH:(bi + 1) * OFF_CH],
                              off_cm[:, b * 128:(b + 1) * 128],
                              ident[:OFF_CH, :OFF_CH])
                      nc.scalar.copy(
                          T[s]["off_pm"][:, b0:b0 + gsz, :].rearrange("p a b -> p (a b)"),
                          pofft[:, 0:gsz * OFF_CH])

                  def emit_ph12(s, gb0, BPG):
                      for ch in range(gb0 // 4, (gb0 + BPG) // 4):
                          emit_ph1_chunk(s, ch)
                      for g8 in range(0, BPG, 8):
                          emit_ph2_group(s, gb0 + g8, min(8, BPG - g8))

                  def emit_ph34(s, gb0, BPG):
                      off_pm, nxy, frac, f0, w0, nxyi, w4, tokf, idxw = (
                          T[s]["off_pm"], T[s]["nxy"], T[s]["frac"], T[s]["f0"],
                          T[s]["w0"], T[s]["nxyi"], T[s]["w4"], T[s]["tokf"], T[s]["idxw"])
                      sl = slice(gb0, gb0 + BPG)
                      # -------- phase 3: prep (DVE) on the granule ----------
                      rampb = ramp[:, sl, None, :].broadcast_to([128, BPG, K2, 2])
                      nc.vector.tensor_tensor(
                          nxy[:, sl].rearrange("p b (k t) -> p b k t", t=2),
                          off_pm[:, sl].rearrange("p b (k t) -> p b k t", t=2),
                          rampb, Alu.add)
                      # clip(nxy, 0, 63) as two ACT relus (dedicated ports):
                      # nxy = relu(63 - relu(63 - nxy)); frac is a free temp.
                      nc.scalar.activation(frac[:, sl], nxy[:, sl], Act.Relu, bias=c63[:], scale=-1.0)
                      nc.scalar.activation(nxy[:, sl], frac[:, sl], Act.Relu, bias=c63[:], scale=-1.0)
                      # floor(nxy): the f32->int cast lands within 1 of the
                      # floor (round or trunc), then subtract 1 where it
                      # went up. Casts run on ACT (dedicated SBUF ports, no
                      # shared-pair contention with SWDGE gather gen).
                      nc.scalar.copy(nxyi[:, sl], nxy[:, sl])
                      nc.scalar.copy(f0[:, sl], nxyi[:, sl])
                      nc.vector.tensor_tensor(frac[:, sl], f0[:, sl], nxy[:, sl], Alu.is_gt)
                      nc.vector.tensor_tensor(f0[:, sl], f0[:, sl], frac[:, sl], Alu.subtract)
                      nc.vector.tensor_tensor(frac[:, sl], nxy[:, sl], f0[:, sl], Alu.subtract)
                      nc.vector.tensor_scalar(w0[:, sl], f0[:, sl], 1.0, 63.0, Alu.add, Alu.min)
                      nc.vector.tensor_tensor(w0[:, sl], w0[:, sl], nxy[:, sl], Alu.subtract)

                      w0v = w0[:, sl].rearrange("p b (k t) -> p b k t", t=2)
                      w1v = frac[:, sl].rearrange("p b (k t) -> p b k t", t=2)
                      w4q = w4[:, sl]
                      # corners: 0=(y0,x0) 1=(y0,x1) 2=(y1,x0) 3=(y1,x1)
                      nc.vector.tensor_tensor(w4q[:, :, :, 0], w0v[:, :, :, 0], w0v[:, :, :, 1], Alu.mult)
                      nc.vector.tensor_tensor(w4q[:, :, :, 1], w1v[:, :, :, 0], w0v[:, :, :, 1], Alu.mult)
                      nc.vector.tensor_tensor(w4q[:, :, :, 2], w0v[:, :, :, 0], w1v[:, :, :, 1], Alu.mult)
                      nc.vector.tensor_tensor(w4q[:, :, :, 3], w1v[:, :, :, 0], w1v[:, :, :, 1], Alu.mult)

                      f0v = f0[:, sl].rearrange("p b (k t) -> p b k t", t=2)
                      nc.vector.scalar_tensor_tensor(
                          tokf[:, sl], f0v[:, :, :, 1], 64.0, f0v[:, :, :, 0],
                          Alu.mult, Alu.add)

                      # -------- phase 4: wrap indices via selector matmuls --
                      # (at <=8-block granularity to bound tile sizes)
                      for qq in range(0, BPG, 8):
                          b0 = gb0 + qq
                          qsz = min(8, BPG - qq)
                          sl8 = slice(b0, b0 + qsz)
                          idxf = pool.tile([16, 8, K2, 8], f32, tag="idxf", bufs=2, name=f"idxf{rep}_{s}_{gb0}_{qq}")
                          for j16 in range(8):
                              psel = psum.tile([16, 8 * K2], f32, tag="psel", bufs=1, name=f"psel{rep}_{s}_{gb0}_{qq}_{j16}")
                              pselv = psel[:, 0:qsz * K2]
                              nc.tensor.matmul(pselv, sel[:, j16, :],
                                               tokf[:, sl8].rearrange("p b k -> p (b k)"),
                                               start=True, stop=True)
                              nc.scalar.copy(
                                  idxf[:, 0:qsz, :, j16],
                                  pselv.rearrange("p (b k) -> p b k", k=K2))
                          idx16 = pool.tile([16, 8, K2, 8], i16, tag="idx16", bufs=2, name=f"idx16{rep}_{s}_{gb0}_{qq}")
                          nc.scalar.copy(idx16[:, 0:qsz], idxf[:, 0:qsz])
                          for r in range(8):
                              nc.sync.dma_start(idxw[16 * r:16 * (r + 1), sl8, :, :], idx16[:, 0:qsz])

                  def emit_chunk(s, bb):
                      # ------- phase 5: gather / combine / matmul -----------
                      w4 = T[s]["w4"]
                      idxw = T[s]["idxw"]
                      tokv = tok_d[s * NPIX:(s + 1) * NPIX, :]
                      g = pool.tile([128, 18, 256], bf, tag="g", bufs=5, name=f"g{rep}_{s}_{bb}")
                      if variant in ("full", "noround"):
                          nc.gpsimd.dma_gather(
                              g[:], tokv,
                              idxw[:, 2 * bb:2 * bb + 2, :, :].rearrange("p b k j -> p (b k j)"),
                              2304, 2304, elem_size=256, single_packet=False,
                              queue_num=bb % 4)
                      else:
                          nc.vector.tensor_copy(g.rearrange("p a b -> p (a b)"), t2full[:])
                      # token layout: [64 c, 4 corners] (corner-minor) so the
                      # weighted mult keeps a stride-1 last dim (2x DVE mode)
                      t = pool.tile([128, 18, 64, 4], bf, tag="t", bufs=1, name=f"t{rep}_{s}_{bb}")
                      if variant != "nocombine":
                          w4s = w4[:, 2 * bb:2 * bb + 2, :, None, :] \
                              .broadcast_to([128, 2, K2, 64, 4]) \
                              .rearrange("p b k c a -> p (b k) c a")
                          nc.vector.tensor_tensor(
                              t[:], g.rearrange("p r (c a) -> p r c a", a=4), w4s, Alu.mult)
                      st = pool.tile([128, 18, 64], bf, tag="st", bufs=2, name=f"st{rep}_{s}_{bb}")
                      u2 = pool.tile([128, 18, 64, 2], bf, tag="u2", bufs=1, name=f"u2{rep}_{s}_{bb}")
                      if variant == "nocombine":
                          nc.vector.tensor_copy(st[:], g.rearrange("p r (c a) -> p r c a", a=4)[:, :, :, 0])
                      else:
                          nc.vector.tensor_tensor(u2[:], t[:, :, :, 0:2], t[:, :, :, 2:4], Alu.add)
                          nc.vector.tensor_tensor(st[:], u2[:, :, :, 0], u2[:, :, :, 1], Alu.add)

                      if variant == "nope5":
                          nc.sync.dma_start(
                              out_d[s, :, bb * 144: bb * 144 + 144],
                              st.bitcast(f32).rearrange("p a b -> p (a b)")[:, 0:144])
                          return
                      for half in range(2):    # the two 128-pixel blocks
                          b = 2 * bb + half
                          pt5 = psum.tile([128, 5, 128], bf, tag="pt5", bufs=2, name=f"pt5_{rep}_{s}_{b}")
                          for kk in range(4):  # paired taps: one [128,128] transpose
                              nc.tensor.transpose(
                                  pt5[:, kk, :],
                                  st[:, half * K2 + 2 * kk: half * K2 + 2 * kk + 2, :]
                                    .rearrange("p a b -> p (a b)"),
                                  identb[:])
                          nc.tensor.transpose(pt5[0:64, 4, :], st[:, half * K2 + 8, :], identb[:])
                          nc.tensor.transpose(pt5[64:128, 4, :], st[:, half * K2 + 8, :], identb[:])
                          samp = pool.tile([128, 5, 128], bf, tag="samp", bufs=3, name=f"samp{rep}_{s}_{b}")
                          nc.scalar.copy(samp.rearrange("p a b -> p (a b)"),
                                         pt5.rearrange("p a b -> p (a b)"))
                          pm = psum.tile([F, 128], f32, tag="pm", bufs=2, name=f"pm{rep}_{s}_{b}")
                          for kk in range(5):
                              nc.tensor.matmul(pm[:], wconv[:, kk, :], samp[:, kk, :],
                                               start=(kk == 0), stop=(kk == 4))
                          osb = pool.tile([F, 128], f32, tag="osb", bufs=3, name=f"osb{rep}_{s}_{b}")
                          nc.scalar.activation(osb[:], pm[:], Act.Identity, bias=bconv[:], scale=1.0)
                          nc.sync.dma_start(out_d[s, :, b * 128:(b + 1) * 128], osb[:])

                  # ---- emission order (best measured): s0 prep pipelined
                  # in granules with a tiny first granule; s1 prep as one
                  # batch; then the two gather/combine streams.
                  load_x(0)
                  load_x(1)
                  for (gb0, BPG) in [(0, 4), (4, 12), (16, 16)]:
                      emit_ph12(0, gb0, BPG)
                      emit_ph34(0, gb0, BPG)
                  emit_ph12(1, 0, NBLK)
                  emit_ph34(1, 0, NBLK)
                  for bb in range(16):
                      emit_chunk(0, bb)
                  for bb in range(16):
                      emit_chunk(1, bb)
    nc.compile()
    return nc


def stage_inputs(x, w_off, b_off, w_conv, b_conv):
    """Host-side staging of all DRAM tensors. Returns per-core in_maps."""
    import ml_dtypes
    bfnp = ml_dtypes.bfloat16
    x = np.ascontiguousarray(x, dtype=np.float32)

    # split precision: x = xh + xl
    xh = x.astype(bfnp).astype(np.float32)
    xl = x - xh

    # channel-major padded images, stacked [xh (0:64); xl (64:128)]
    xcm = np.zeros((B, 128, 66, 66), np.float32)
    xcm[:, :C, 1:65, 1:65] = np.transpose(xh, (0, 3, 1, 2))
    xcm[:, C:2 * C, 1:65, 1:65] = np.transpose(xl, (0, 3, 1, 2))
    xcm = xcm.reshape(B, 128, 66 * 66).astype(bfnp)

    # xcm2: [xh; xh shifted up one image row] for the (t, t+3) low-pair
    # matmuls (window at row r reads tap t rows on 0:64 and tap t+3 rows
    # = r+1 on 64:128).
    xcm2 = np.zeros((B, 128, 66 * 66), np.float32)
    xcm2[:, :C] = xcm[:, :C].astype(np.float32)
    xcm2[:, C:, :66 * 65] = xcm[:, :C, 66:].astype(np.float32)
    xcm2 = xcm2.astype(bfnp)

    # token table: [B*4096, 256] bf16 patches, corner-minor layout [64c, 4j]
    xp = np.zeros((B, H + 1, W + 1, C), np.float32)
    xp[:, :H, :W] = x
    tok = np.empty((B, H, W, C, 4), np.float32)
    tok[:, :, :, :, 0] = xp[:, :H, :W]
    tok[:, :, :, :, 1] = xp[:, :H, 1:W + 1]
    tok[:, :, :, :, 2] = xp[:, 1:H + 1, :W]
    tok[:, :, :, :, 3] = xp[:, 1:H + 1, 1:W + 1]
    tok = tok.reshape(B * NPIX, 256).astype(bfnp)

    # offset conv weights: wh duplicated on both K halves; wl split into
    # pairs (t, t+3) on stacked partitions + singles (6, 7, 8)
    wh = w_off.astype(bfnp).astype(np.float32)
    wl = (w_off - wh).astype(bfnp).astype(np.float32)
    woff = np.zeros((128, K2, OFF_CH), np.float32)
    for tap in range(K2):
        woff[:C, tap] = wh[tap // 3, tap % 3]
        woff[C:, tap] = wh[tap // 3, tap % 3]
    woff = woff.astype(bfnp)
    wlp = np.zeros((128, 3, OFF_CH), np.float32)
    wloff = np.zeros((64, 3, OFF_CH), np.float32)
    for t in range(3):
        wlp[:C, t] = wl[0, t]          # tap t = (dy=0, dx=t)
        wlp[C:, t] = wl[1, t]          # tap t+3 = (dy=1, dx=t)
        wloff[:, t] = wl[2, t]         # tap t+6 = (dy=2, dx=t)
    wlp = wlp.astype(bfnp)
    wloff = wloff.astype(bfnp)

    # stacked k-pair conv weights [128, 5, 128]
    wc = w_conv.reshape(K2, C, F)
    wconv5 = np.zeros((128, 5, F), np.float32)
    for kk in range(4):
        wconv5[0:64, kk] = wc[2 * kk]
        wconv5[64:128, kk] = wc[2 * kk + 1]
    wconv5[0:64, 4] = wc[8]
    wconv5 = wconv5.astype(bfnp)

    pidx = np.arange(NPIX)
    ramp = np.stack([(pidx % W).astype(np.float32), (pidx // W).astype(np.float32)], -1)
    ramp = ramp.reshape(NBLK, 128, 2).transpose(1, 0, 2).copy()  # [128, NBLK, 2]

    selm = np.zeros((128, 8, 16), np.float32)
    for p in range(128):
        selm[p, p // 16, p % 16] = 1.0

    boff = np.ascontiguousarray(b_off, dtype=np.float32).reshape(OFF_CH, 1)
    bconv = np.ascontiguousarray(b_conv, dtype=np.float32).reshape(F, 1)

    in_maps = []
    for i in range(NCORES):
        sl = slice(SPB * i, SPB * (i + 1))
        in_maps.append({
            "xcm": np.ascontiguousarray(xcm[sl]),
            "xcm2": np.ascontiguousarray(xcm2[sl]),
            "tok": np.ascontiguousarray(tok[SPB * i * NPIX: SPB * (i + 1) * NPIX]),
            "woff": woff, "wloff": wloff, "wlp": wlp, "wconv5": wconv5,
            "boff": boff, "bconv": bconv, "ramp": ramp, "sel": selm,
        })
    return in_maps


_NC_CACHE = {}


def get_nc():
    if "nc" not in _NC_CACHE:
        _NC_CACHE["nc"] = build_nc()
    return _NC_CACHE["nc"]


def kernel(x, w_off, b_off, w_conv, b_conv):
    from concourse.bass_utils import run_bass_kernel_spmd
    nc = get_nc()
    in_maps = stage_inputs(np.asarray(x), np.asarray(w_off), np.asarray(b_off),
                           np.asarray(w_conv), np.asarray(b_conv))
    res = run_bass_kernel_spmd(nc, in_maps, core_ids=list(range(NCORES)))
    out = np.empty((B, H * W, F), np.float32)
    for i in range(NCORES):
        o = res.results[i]["out"]          # [SPB, F, NPIX]
        for s in range(SPB):
            out[SPB * i + s] = np.asarray(o[s]).T
    return out.reshape(B, H, W, F)
